# revision 1
# baseline (speedup 1.0000x reference)
"""PASA downsample (group softmax) Trainium2 kernel.

Math (per batch image, all per-core):
  xp  = reflect_pad(x, 1)                                  [64, 130, 130]
  sig = conv3x3(xp, w, stride=2)  (+ BN inference, folded) [72, 64, 64]
  e   = exp(sig)                                           [72, 64, 64]
  Z   = sum_ch e                                           [1, 64, 64]
  out[c] = (sum_k e[g(c)*9+k] * xp[c, 2i+kh, 2j+kw]) / Z   [64, 64, 64]

Sharding: data-parallel over batch (8 images -> 8 cores), params replicated.

On-chip layout: partitions = (row_half, channel): partition 64*h + c.
x is staged band-by-band into a packed buffer (contiguous 4KB DMA
descriptors = full DMA rate), then split into two column-parity planes:
  PA[p, r, j] = x[c, 64h+r-1, 2j-1]  (odd x cols; col 0 = reflect dup)
  PB[p, r, j] = x[c, 64h+r-1, 2j]    (even x cols)
so every conv-matmul rhs and every combine operand is unit-stride
(2-byte + unit-stride + SBUF => DVE 2x mode for the combine mul/adds).
Stride-2 conv tap (kh, kw) at out row i reads plane row 2i+kh:
  kw=0 -> PA[:, :, 0:64], kw=1 -> PB[:, :, 0:64], kw=2 -> PA[:, :, 1:65].

e replication across each group's 8 channels is a stride-0 broadcast
SBUF->SBUF DMA (bf16) instead of PE matmuls: PE does only conv + Z.
"""

import numpy as np
import ml_dtypes
from contextlib import ExitStack

import concourse.bass as bass
import concourse.bacc as bacc_mod
import concourse.mybir as mybir
import concourse.tile as tile
from concourse.bass_utils import run_bass_kernel_spmd

EPS = 1e-5
G = 8
N_CORES = 8

F32 = mybir.dt.float32
BF16 = mybir.dt.bfloat16
NP_BF16 = ml_dtypes.bfloat16

DT = BF16
NP_DT = NP_BF16

# tap split between engines for the combine (9 taps total), per quarter-pair
DVE_TAPS = {0: (0, 1, 2, 3, 4), 1: (0, 1, 2, 3, 4, 5)}
POOL_TAPS = {0: (5, 6, 7, 8), 1: (6, 7, 8)}
# e_rep broadcast issue plan: (engine, tap) in queue order.  qp0's Act
# share is deferred into the q=2 block so it can't delay exp q2 (PSUM).
EREP_SP = {0: (5, 0, 6, 1, 7, 2), 1: (6, 0, 7, 1, 2)}
EREP_ACT = {0: (3, 4, 8), 1: (8, 3, 4, 5)}
# conv tap order: taps needing only plane interiors first (PB, PA[1:]),
# pad-dependent taps (col 0 / row 0) last, so conv starts off partial planes
CONV_TAP_ORDER = (4, 7, 5, 8, 3, 6, 1, 2, 0)


def build_bass(bench_iters=0):
    nc = bacc_mod.Bacc("TRN2", target_bir_lowering=False, debug=False,
                       num_swdge_queues=2)
    x_d = nc.dram_tensor("x", [64, 128, 128], F32, kind="ExternalInput")
    wt_d = nc.dram_tensor("wt", [64, 9, 72], DT, kind="ExternalInput")
    bnb_d = nc.dram_tensor("bnb", [72, 1], F32, kind="ExternalInput")
    ones_d = nc.dram_tensor("ones", [72, 64], DT, kind="ExternalInput")
    out_d = nc.dram_tensor("out", [64, 64, 64], F32, kind="ExternalOutput")

    with ExitStack() as ctx:
        tc = ctx.enter_context(tile.TileContext(nc))
        const = ctx.enter_context(tc.tile_pool(name="const", bufs=1))
        big = ctx.enter_context(tc.tile_pool(name="big", bufs=1))
        xrawp = ctx.enter_context(tc.tile_pool(name="xraw", bufs=2))
        prodp = ctx.enter_context(tc.tile_pool(name="prod", bufs=4))
        psig = ctx.enter_context(tc.tile_pool(name="psig", bufs=3, space="PSUM"))
        pz = ctx.enter_context(tc.tile_pool(name="pz", bufs=2, space="PSUM"))

        wt_sb = const.tile([128, 9, 72], DT)  # weights duplicated on both halves
        bnb_sb = const.tile([72, 1], F32)
        ones_sb = const.tile([72, 64], DT)
        nc.sync.dma_start(out=wt_sb[0:64], in_=wt_d[:])
        nc.sync.dma_start(out=wt_sb[64:128], in_=wt_d[:])
        nc.sync.dma_start(out=bnb_sb, in_=bnb_d[:])
        nc.sync.dma_start(out=ones_sb, in_=ones_d[:])

        pa = big.tile([128, 65, 65], DT)   # odd x cols (col 0 = reflect dup)
        pb = big.tile([128, 65, 64], DT)   # even x cols
        e_sb = big.tile([72, 4096], DT)
        e_rep = big.tile([128, 9, 2048], DT)
        rr = big.tile([128, 2048], DT)
        acc_d = big.tile([128, 32, 64], DT)
        acc_p = big.tile([128, 32, 64], DT)
        out_sb = big.tile([128, 32, 64], F32)

        import contextlib
        loop_cm = tc.For_i(0, bench_iters, 1) if bench_iters else contextlib.nullcontext()
        with loop_cm:
            body_pipeline(nc, x_d, out_d, xrawp, pa, pb, e_sb, e_rep, rr,
                          acc_d, acc_p, out_sb, wt_sb, bnb_sb, ones_sb,
                          psig, pz, prodp)

    nc.finalize()
    return nc


def _ap(base_ap, extra_off, dims):
    return bass.AP(tensor=base_ap.tensor, offset=base_ap.offset + extra_off,
                   ap=dims)


def band_load(nc, x_d, xraw, q):
    """DMA x band q (packed, both halves in one AP) into xraw staging."""
    xb = x_d[:]
    if q == 0:
        # h0: x rows 0..15 -> slots 1..16 ; h1: x rows 63..79 -> slots 0..16
        nc.gpsimd.dma_start(out=xraw[0:64, 1:17, :], in_=x_d[:, 0:16, :])
        nc.gpsimd.dma_start(out=xraw[64:128, 0:17, :], in_=x_d[:, 63:80, :])
    else:
        # both halves: x rows 64h+16q .. +15 -> slots 0..15
        src = _ap(xb, 16 * q * 128,
                  [[64 * 128, 2], [128 * 128, 64], [128, 16], [1, 128]])
        nc.gpsimd.dma_start(out=xraw[:, 0:16, :], in_=src)


def band_planes(nc, xraw, pa, pb, q):
    """Split band q staging into column-parity planes (PA on Act, PB on DVE)."""
    if q == 0:
        # per-half builds so h0's conv can start before h1's DMA lands
        for h0 in range(2):
            p = slice(64 * h0, 64 * h0 + 64)
            nc.vector.tensor_copy(pb[p, 1:17, 0:64], xraw[p, 1:17, 0:128:2])
            nc.scalar.copy(pa[p, 1:17, 1:65], xraw[p, 1:17, 1:128:2])
            nc.scalar.copy(pa[p, 1:17, 0:1], xraw[p, 1:17, 1:2])
            # row 0: h0 = reflect x row 1 (slot 2); h1 = x row 63 (slot 0)
            s = 2 - 2 * h0
            nc.vector.tensor_copy(pb[p, 0:1, 0:64], xraw[p, s:s + 1, 0:128:2])
            nc.scalar.copy(pa[p, 0:1, 1:65], xraw[p, s:s + 1, 1:128:2])
            nc.scalar.copy(pa[p, 0:1, 0:1], xraw[p, s:s + 1, 1:2])
    else:
        r0 = 16 * q + 1
        nc.scalar.copy(pa[:, r0:r0 + 16, 1:65], xraw[:, 0:16, 1:128:2])
        nc.vector.tensor_copy(pb[:, r0:r0 + 16, 0:64], xraw[:, 0:16, 0:128:2])
        nc.scalar.copy(pa[:, r0:r0 + 16, 0:1], xraw[:, 0:16, 1:2])


def tap_view(pa, pb, t9, row0, nrows, part=slice(0, 128)):
    """Unit-stride plane view for tap t9 starting at plane row row0."""
    kh, kw = divmod(t9, 3)
    pl = pb if kw == 1 else pa
    c0 = 1 if kw == 2 else 0
    return pl[part, row0 + kh:row0 + kh + 2 * nrows - 1:2, c0:c0 + 64]


def body_pipeline(nc, x_d, out_d, xrawp, pa, pb, e_sb, e_rep, rr,
                  acc_d, acc_p, out_sb, wt_sb, bnb_sb, ones_sb,
                  psig, pz, prodp):
    est = e_sb.ap[0][0]  # e_sb partition stride (elements)
    rr3 = rr.rearrange("p (a b) -> p a b", a=32)
    stores = []

    for q in range(4):
        # ---- stage band q, build planes ----
        xraw = xrawp.tile([128, 17, 128], DT)
        band_load(nc, x_d, xraw, q)
        band_planes(nc, xraw, pa, pb, q)

        # ---- conv (9 taps, contraction 64) + BN bias + exp ----
        for h0 in range(2):
            ps = psig.tile([72, 512], F32)
            for i, t9 in enumerate(CONV_TAP_ORDER):
                rhs = tap_view(pa, pb, t9, 16 * q, 8,
                               part=slice(64 * h0, 64 * h0 + 64))
                nc.tensor.matmul(ps, lhsT=wt_sb[64 * h0:64 * h0 + 64, t9, :],
                                 rhs=rhs, start=(i == 0), stop=(i == 8))
            col0 = 2048 * h0 + 512 * q
            nc.scalar.activation(
                out=e_sb[:, col0:col0 + 512], in_=ps,
                func=mybir.ActivationFunctionType.Exp,
                bias=bnb_sb, scale=1.0,
            )

        # ---- Z (replicated to 128 partitions via ones-matmul) + recip ----
        pzt = pz.tile([128, 512], F32)
        nc.tensor.matmul(pzt[0:64, :], lhsT=ones_sb,
                         rhs=e_sb[:, 512 * q:512 * q + 512],
                         start=True, stop=True)
        nc.tensor.matmul(pzt[64:128, :], lhsT=ones_sb,
                         rhs=e_sb[:, 2048 + 512 * q:2048 + 512 * q + 512],
                         start=True, stop=True)
        with nc.allow_low_precision(reason="bf16 recip feeds bf16 combine"):
            nc.vector.reciprocal(out=rr[:, 512 * q:512 * q + 512], in_=pzt)

        # e-replication broadcasts: src partition 8t+g -> dst 8g..8g+7, per
        # (tap, half).  SP takes the early-needed taps at each qp; Act's
        # qp0 share is emitted later (see below) to keep exp unblocked.
        def erep_issue(eng, taps, qp):
            c0 = 1024 * qp
            for t9 in taps:
                for h0 in range(2):
                    src = _ap(e_sb[:], 8 * t9 * est + 2048 * h0 + c0,
                              [[est, 8], [0, 8], [1, 1024]])
                    eng.dma_start(
                        out=e_rep[64 * h0:64 * h0 + 64, t9, c0:c0 + 1024],
                        in_=src)

        if q == 1:
            erep_issue(nc.sync, EREP_SP[0], 0)
            continue
        if q == 0:
            continue

        # ---- quarter-pair combine, emitted one quarter late so the Act
        # share of the broadcasts lands after exp q2 in Act's queue ----
        qp = q - 2
        c0 = 1024 * qp
        erep_issue(nc.scalar, EREP_ACT[qp], qp)
        if qp == 1:
            erep_issue(nc.sync, EREP_SP[1], 1)

        row0 = 32 * qp
        ad = acc_d[:, 16 * qp:16 * qp + 16, :]
        apc = acc_p[:, 16 * qp:16 * qp + 16, :]
        for eng, taps, accv in ((nc.vector, DVE_TAPS[qp], ad),
                                (nc.gpsimd, POOL_TAPS[qp], apc)):
            for i, t9 in enumerate(taps):
                xv = tap_view(pa, pb, t9, row0, 16)
                ev = e_rep[:, t9, c0:c0 + 1024].rearrange(
                    "p (a b) -> p a b", a=16)
                if i == 0:
                    eng.tensor_mul(accv, xv, ev)
                else:
                    prod = prodp.tile([128, 16, 64], DT)
                    eng.tensor_mul(prod, xv, ev)
                    eng.tensor_add(accv, accv, prod)
        # join the two partial accumulators + normalize; DVE handles qp1's
        # (it is the faster engine and qp1's join is on the critical tail)
        osl = out_sb[:, 16 * qp:16 * qp + 16, :]
        if qp == 0:
            nc.vector.tensor_add(ad, ad, apc)
            nc.gpsimd.tensor_mul(osl, ad, rr3[:, 16 * qp:16 * qp + 16, :])
        else:
            nc.vector.tensor_add(ad, ad, apc)
            nc.vector.tensor_mul(osl, ad, rr3[:, 16 * qp:16 * qp + 16, :])
        for h0 in range(2):
            stores.append((
                out_d[:, 32 * h0 + 16 * qp:32 * h0 + 16 * qp + 16, :],
                out_sb[64 * h0:64 * h0 + 64, 16 * qp:16 * qp + 16, :]))

    # stores at the very end of SP's in-order queue so a slow norm can't
    # head-of-line block later e_rep broadcasts
    for dst, src in stores:
        nc.sync.dma_start(out=dst, in_=src)


def host_prep(conv_w, gamma, beta, running_mean, running_var):
    inv = 1.0 / np.sqrt(np.asarray(running_var, np.float64) + EPS)
    scale = (np.asarray(gamma, np.float64) * inv).astype(np.float32)  # [72]
    bias = (np.asarray(beta, np.float64)
            - np.asarray(running_mean, np.float64) * inv * np.asarray(gamma, np.float64)
            ).astype(np.float32)
    wt = np.asarray(conv_w, np.float32) * scale[:, None, None, None]  # [72,64,3,3]
    # permute output channels from (g, k) to (k, g) order: row 8*k + g
    perm = np.array([g * 9 + k for k in range(9) for g in range(G)])
    wt = wt[perm]
    bias = bias[perm]
    wt = np.ascontiguousarray(wt.transpose(1, 2, 3, 0).reshape(64, 9, 72))
    return {
        "wt": wt.astype(NP_DT),
        "bnb": np.ascontiguousarray(bias.reshape(72, 1)),
        "ones": np.ones((72, 64), NP_DT),
    }


_NC_CACHE = {}


def kernel(x, conv_w, gamma, beta, running_mean, running_var):
    x = np.asarray(x, np.float32)
    n = x.shape[0]
    aux = host_prep(conv_w, gamma, beta, running_mean, running_var)
    if "nc" not in _NC_CACHE:
        _NC_CACHE["nc"] = build_bass()
    nc = _NC_CACHE["nc"]
    in_maps = [dict(aux, x=np.ascontiguousarray(x[i])) for i in range(n)]
    res = run_bass_kernel_spmd(nc, in_maps, core_ids=list(range(n)))
    return np.stack([r["out"] for r in res.results], axis=0)


if __name__ == "__main__":
    rng = np.random.default_rng(0)
    x = rng.standard_normal((8, 64, 128, 128), dtype=np.float32)
    cw = (rng.standard_normal((72, 64, 3, 3)) * np.sqrt(2.0 / (72 * 9))).astype(np.float32)
    out = kernel(x, cw, np.ones(72, np.float32), np.zeros(72, np.float32),
                 np.zeros(72, np.float32), np.ones(72, np.float32))
    print(out.shape, out.dtype)

